# revision 7
# baseline (speedup 1.0000x reference)
"""ColBERT MaxSim retrieval kernel for Trainium2 (8 NeuronCores).

scores[b, n] = sum_{q active} max_{t active} cos(q_hidden[b,q], d_hidden[n,t])

Host: d tokens are masked, L2-normalized, compacted (only active tokens
kept), bf16-cast, and laid out per a schedule that the device program is
specialized to. Query active tokens are packed raw into 128 slots; query
inverse norms fold into the final scores matmul weights.

Device (per core, 128 docs): stream dn through PE (stationary qT, bf16,
512-col matmuls into PSUM). The token-max runs on two lanes:
  - direct:  Vector tensor_reduce [128, G, T] straight from PSUM
  - offload: Scalar copies PSUM->SBUF fp16, then Vector runs a contiguous
    TT-max tree (2 elems/cycle in fp16 2x mode). Host lays the offloaded
    superblocks out in 8-way bit-blocks (col(b,d,j) = b*G*T/8 + d*T/8 + j,
    token = j*8+b) so all three halving levels are single-segment
    contiguous TTs; the tail reduce sees a doc-major [128, G, T/8] view.
Final: PE matmul mx.T @ (onehot*qinv) -> [doc, batch] scores, DMA out.

Stream order = [primer direct tiles (smallest docs), superblocks,
direct tail tiles (largest docs)] so Vector starts early and the last
DMA arrivals take the shortest path. DMA: first descriptor is the small
primer tile; qt/ohw/output ride the Scalar-engine HW queue.
"""

import os
import sys
from contextlib import ExitStack

import numpy as np

sys.path.insert(0, "/opt/trn_rl_repo")

# ---- problem constants (hardcoded per contest contract) ----
B, Lq, N, Ld, K = 8, 32, 1024, 256, 128
NCORES = 8
D = N // NCORES          # 128 docs per core
EPS = 1e-8
NEG = -100.0

TILE_COLS = 1024         # PSUM tile: 2 banks, triple-buffered
MM_CHUNK = 512
PRIMER_TILES = 2         # leading direct tiles (pipeline fill)
TAIL_TILES = 1           # trailing direct tiles (largest-T docs)
SB_COLS = 5120           # superblock target size (5 PSUM tiles)

# measured per-column engine rates (ns) for load balancing
R_TREE = 0.72            # Vector TT tree + tail, per sb col
R_RED = 1.19             # Vector direct reduce, per col (incl per-op init)
R_COPY = 0.99            # Scalar PSUM->SBUF copy, per col

_CACHE = {}
LAST_EXEC_NS = None


def _plan(tpos):
    """tpos: [D] per-sorted-position token requirement (cross-core max).
    Returns (items, c_total). Items in stream order:
      ('direct', G, T, doc_off, col_off, cols)
      ('sb',     G, T, doc_off, col_off, cols)   # cols = G*T, bit-block laid
    """
    items = []
    pos = 0
    end = D

    def t_even(p):
        return int((tpos[p] + 1) // 2 * 2)

    def t_8(p):
        return int((tpos[p] + 7) // 8 * 8)

    def take_direct(pos, end_limit):
        g = 1
        while pos + g < end_limit and (g + 1) * t_even(pos + g) <= TILE_COLS:
            g += 1
        t = t_even(pos + g - 1)
        return ("direct", g, t, pos, None, g * t)

    # tail: reserve the largest docs for one trailing direct tile
    tail = []
    for _ in range(TAIL_TILES):
        g = 1
        t = t_even(end - 1)
        while end - g - 1 >= pos and (g + 1) * t <= TILE_COLS:
            g += 1
        end -= g
        tail.insert(0, ("direct", g, t, end, None, g * t))

    v_load = s_load = 0.0
    for _ in range(PRIMER_TILES):
        it = take_direct(pos, end)
        items.append(it)
        v_load += it[5] * R_RED
        pos += it[1]

    while pos < end:
        # superblock
        g = 1
        while pos + g < end and (g + 1) * t_8(pos + g) <= SB_COLS:
            g += 1
        t = t_8(pos + g - 1)
        it = ("sb", g, t, pos, None, g * t)
        items.append(it)
        v_load += g * t * R_TREE
        s_load += g * t * R_COPY
        pos += g
        # plug a direct tile when Vector is clearly behind Scalar
        if pos < end and s_load - v_load > 1000.0:
            it = take_direct(pos, end)
            items.append(it)
            v_load += it[5] * R_RED
            pos += it[1]

    items.extend(tail)

    col = 0
    out = []
    for kind, g, t, doc_off, _, cols in items:
        out.append((kind, g, t, doc_off, col, cols))
        col += cols
    return out, col


def _build_program(items, c_total, qt_cols):
    import concourse.bacc as bacc
    import concourse.mybir as mybir
    import concourse.tile as tile

    f32 = mybir.dt.float32
    bf16 = mybir.dt.bfloat16
    fp16 = mybir.dt.float16
    AL = mybir.AluOpType
    ACTF = mybir.ActivationFunctionType

    nc = bacc.Bacc("TRN2", target_bir_lowering=False)

    dn = nc.dram_tensor("dn", [K, c_total], bf16, kind="ExternalInput")
    qt = nc.dram_tensor("qt", [K, qt_cols], bf16, kind="ExternalInput")
    oh = nc.dram_tensor("ohw", [128, B], fp16, kind="ExternalInput")
    sc = nc.dram_tensor("scores", [D, B], f32, kind="ExternalOutput")

    with ExitStack() as ctx:
        tc = ctx.enter_context(tile.TileContext(nc))
        const = ctx.enter_context(tc.tile_pool(name="const", bufs=1))
        trees = ctx.enter_context(tc.tile_pool(name="trees", bufs=2))
        pssim = ctx.enter_context(tc.tile_pool(name="pssim", bufs=3, space="PSUM"))
        psout = ctx.enter_context(tc.tile_pool(name="psout", bufs=1, space="PSUM"))

        # small constants ride the Scalar-engine HW DGE queue so the Sync
        # queue's first descriptor is the primer doc tile
        qt_sb = const.tile([K, qt_cols], bf16)
        nc.scalar.dma_start(qt_sb, qt[:, :])
        oh_sb = const.tile([128, B], fp16)
        nc.scalar.dma_start(oh_sb, oh[:, :])

        mx = const.tile([128, D], fp16)

        # per-item SBUF chunk tiles + DMA (sync queue, stream order)
        chunk = {}
        for i, (kind, g, t, doc_off, col_off, cols) in enumerate(items):
            ct = const.tile([K, cols], bf16, name=f"dchunk{i}", tag=f"dchunk{i}")
            nc.sync.dma_start(ct, dn[:, col_off:col_off + cols])
            chunk[i] = ct

        # deferred V-lane work (direct tail reduces emitted after trees)
        def emit_matmuls(ct, cols):
            """PE matmuls for one item; returns list of (psum_tile, lo, sz)."""
            pieces = []
            for p0 in range(0, cols, TILE_COLS):
                psz = min(TILE_COLS, cols - p0)
                sim = pssim.tile([128, TILE_COLS], f32, tag="sim")
                for s in range(0, psz, MM_CHUNK):
                    e = min(s + MM_CHUNK, psz)
                    nc.tensor.matmul(
                        sim[:, s:e], qt_sb, ct[:, p0 + s:p0 + e],
                        start=True, stop=True, skip_group_check=True,
                    )
                pieces.append((sim, p0, psz))
            return pieces

        for i, (kind, g, t, doc_off, col_off, cols) in enumerate(items):
            ct = chunk[i]
            pieces = emit_matmuls(ct, cols)
            if kind == "direct":
                (sim, _, psz) = pieces[0]
                nc.vector.tensor_reduce(
                    mx[:, doc_off:doc_off + g],
                    sim[:, :psz].rearrange("p (d t) -> p d t", d=g),
                    axis=mybir.AxisListType.X, op=AL.max,
                )
            else:
                arena = trees.tile([128, cols], fp16, name=f"arena{i}",
                                   tag="arena")
                for (sim, p0, psz) in pieces:
                    nc.scalar.activation(
                        arena[:, p0:p0 + psz], sim[:, :psz], ACTF.Copy,
                        bias=0.0, scale=1.0,
                    )
                h = cols // 2
                t1 = trees.tile([128, h], fp16, name=f"t1_{i}", tag="t1")
                nc.vector.tensor_max(t1, arena[:, :h], arena[:, h:])
                t2 = trees.tile([128, h // 2], fp16, name=f"t2_{i}", tag="t2")
                nc.vector.tensor_max(t2, t1[:, :h // 2], t1[:, h // 2:])
                t3 = trees.tile([128, h // 4], fp16, name=f"t3_{i}", tag="t3")
                nc.vector.tensor_max(t3, t2[:, :h // 4], t2[:, h // 4:])
                nc.vector.tensor_reduce(
                    mx[:, doc_off:doc_off + g],
                    t3.rearrange("p (d t) -> p d t", d=g),
                    axis=mybir.AxisListType.X, op=AL.max,
                )

        scp = psout.tile([D, B], f32, tag="scp")
        nc.tensor.matmul(scp, mx, oh_sb, start=True, stop=True,
                         skip_group_check=True)
        scsb = const.tile([D, B], f32)
        nc.vector.tensor_copy(scsb, scp)
        nc.scalar.dma_start(sc[:, :], scsb)

    nc.finalize()
    return nc


def _get_program(items, c_total, qt_cols):
    key = (tuple(items), c_total, qt_cols)
    if key not in _CACHE:
        _CACHE[key] = _build_program(items, c_total, qt_cols)
    return _CACHE[key]


def kernel(q_hidden, q_mask, d_hidden, d_mask):
    global LAST_EXEC_NS
    import ml_dtypes
    from concourse.bass_utils import run_bass_kernel_spmd

    bf16 = ml_dtypes.bfloat16
    q_hidden = np.asarray(q_hidden, dtype=np.float32)
    q_mask = np.asarray(q_mask)
    d_hidden = np.asarray(d_hidden, dtype=np.float32)
    d_mask = np.asarray(d_mask)

    # ---- doc prep ----
    dnorm = np.sqrt(np.sum(d_hidden * d_hidden, axis=-1, keepdims=True))
    dhat = d_hidden / np.maximum(dnorm, EPS)               # [N, Ld, K]
    act_d = d_mask > 0
    n_act = act_d.sum(axis=1).astype(np.int64)             # [N]

    orders = []
    sorted_nact = np.empty((NCORES, D), np.int64)
    for c in range(NCORES):
        na = n_act[c * D:(c + 1) * D]
        o = np.argsort(na, kind="stable")
        orders.append(o)
        sorted_nact[c] = na[o]

    tpos = np.maximum(sorted_nact.max(axis=0), 2)
    items, c_total = _plan(tpos)

    # token->column maps per core
    tok_idx = np.zeros((NCORES, c_total), np.int64)
    doc_idx = np.zeros((NCORES, c_total), np.int64)
    for c in range(NCORES):
        for (kind, g, t, doc_off, col_off, cols) in items:
            for j in range(g):
                doc = c * D + int(orders[c][doc_off + j])
                ti = np.nonzero(act_d[doc])[0]
                reps = np.zeros(t, np.int64)
                if len(ti):
                    k = min(len(ti), t)
                    reps[:k] = ti[:k]
                    reps[k:] = ti[0]
                if kind == "direct":
                    doc_idx[c, col_off + j * t:col_off + (j + 1) * t] = doc
                    tok_idx[c, col_off + j * t:col_off + (j + 1) * t] = reps
                else:
                    # bit-block: col(b, d, jj) = b*(g*t/8) + d*(t/8) + jj,
                    # token index = jj*8 + b
                    t8 = t // 8
                    for b in range(8):
                        s = col_off + b * g * t8 + j * t8
                        tok_idx[c, s:s + t8] = reps[b::8][:t8]
                        doc_idx[c, s:s + t8] = doc
    dn_all = dhat[doc_idx, tok_idx]                        # [NCORES, C, K]
    dn_all = np.ascontiguousarray(dn_all.transpose(0, 2, 1)).astype(bf16)

    # ---- query prep ----
    qf = q_hidden.reshape(B * Lq, K)
    act = np.nonzero(q_mask.reshape(-1) > 0)[0]
    nq = len(act)
    assert nq <= 128, f"active q tokens {nq} > 128 unsupported"
    QT = 128
    qpack = np.ones((QT, K), np.float32)
    qpack[:nq] = qf[act]
    qn = np.sqrt(np.sum(qpack * qpack, axis=-1))
    qinv = 1.0 / np.maximum(qn, EPS)
    ohw = np.zeros((QT, B), np.float32)
    ohw[np.arange(nq), act // Lq] = qinv[:nq]
    ohw = ohw.astype(np.float16)
    qt_p = np.ascontiguousarray(qpack.T).astype(bf16)      # [K, QT]

    in_maps = [
        {"dn": dn_all[c], "qt": qt_p, "ohw": ohw} for c in range(NCORES)
    ]

    nc = _get_program(items, c_total, QT)
    kw = {}
    if os.environ.get("KRN_TMPDIR"):
        kw["tmpdir"] = os.environ["KRN_TMPDIR"]
    br = run_bass_kernel_spmd(nc, in_maps, core_ids=list(range(NCORES)), **kw)
    if br.exec_time_ns is not None:
        LAST_EXEC_NS = br.exec_time_ns

    scores = np.empty((B, N), np.float32)
    for c in range(NCORES):
        out_c = br.results[c]["scores"]                    # [D, B] sorted order
        scores[:, c * D + orders[c]] = out_c.T
    dead = np.nonzero(n_act == 0)[0]
    if len(dead):
        nactq = (q_mask > 0).sum(axis=1).astype(np.float32)
        scores[:, dead] = (NEG * nactq)[:, None]
    return scores


if __name__ == "__main__":
    tpos = np.full(D, 129)
    items, c_total = _plan(tpos)
    for it in items:
        print(it)
    print("c_total:", c_total)
    nc = _get_program(tuple(items), c_total, 128)
    print("program built OK; instructions:",
          sum(len(bb.instructions) for bb in nc.main_func.blocks))


# revision 10
# speedup vs baseline: 1.0618x; 1.0618x over previous
"""ColBERT MaxSim retrieval kernel for Trainium2 (8 NeuronCores).

scores[b, n] = sum_{q active} max_{t active} cos(q_hidden[b,q], d_hidden[n,t])

Host: d tokens are masked, L2-normalized, compacted (only active tokens
kept), bf16-cast, and laid out per a schedule that the device program is
specialized to. Query active tokens are packed raw into 128 slots; query
inverse norms fold into the final scores matmul weights.

Device (per core, 128 docs): stream dn through PE (stationary qT, bf16,
512-col matmuls into PSUM). The token-max runs on two lanes:
  - direct:  Vector tensor_reduce [128, G, T] straight from PSUM
  - offload: Scalar copies PSUM->SBUF fp16, then Vector runs a contiguous
    TT-max tree (2 elems/cycle in fp16 2x mode). Host lays the offloaded
    superblocks out in 8-way bit-blocks (col(b,d,j) = b*G*T/8 + d*T/8 + j,
    token = j*8+b) so all three halving levels are single-segment
    contiguous TTs; the tail reduce sees a doc-major [128, G, T/8] view.
Final: PE matmul mx.T @ (onehot*qinv) -> [doc, batch] scores, DMA out.

Stream order = [primer direct tiles (smallest docs), superblocks,
direct tail tiles (largest docs)] so Vector starts early and the last
DMA arrivals take the shortest path. DMA: first descriptor is the small
primer tile; qt/ohw/output ride the Scalar-engine HW queue.
"""

import os
import sys
from contextlib import ExitStack

import numpy as np

sys.path.insert(0, "/opt/trn_rl_repo")

# ---- problem constants (hardcoded per contest contract) ----
B, Lq, N, Ld, K = 8, 32, 1024, 256, 128
NCORES = 8
D = N // NCORES          # 128 docs per core
EPS = 1e-8
NEG = -100.0

TILE_COLS = 1024         # PSUM tile: 2 banks, triple-buffered
MM_CHUNK = 512
PRIMER_TILES = 2         # leading direct tiles (pipeline fill)
TAIL_TILES = 1           # trailing direct tiles (largest-T docs)
SB_COLS = 5120           # superblock target size (5 PSUM tiles)

# measured per-column engine rates (ns) for load balancing
R_TREE = 0.72            # Vector TT tree + tail, per sb col
R_RED = 1.19             # Vector direct reduce, per col (incl per-op init)
R_COPY = 1.08            # Scalar PSUM->SBUF copy, per col (1024-col pieces)

_CACHE = {}
LAST_EXEC_NS = None


def _plan(tpos):
    """tpos: [D] per-sorted-position token requirement (cross-core max).
    Returns (items, c_total). Items in stream order:
      ('direct', G, T, doc_off, col_off, cols)
      ('sb',     G, T, doc_off, col_off, cols)   # cols = G*T, bit-block laid
    """
    items = []
    pos = 0
    end = D

    def t_even(p):
        return int((tpos[p] + 1) // 2 * 2)

    def t_8(p):
        return int((tpos[p] + 7) // 8 * 8)

    def take_direct(pos, end_limit):
        g = 1
        while pos + g < end_limit and (g + 1) * t_even(pos + g) <= TILE_COLS:
            g += 1
        t = t_even(pos + g - 1)
        return ("direct", g, t, pos, None, g * t)

    # tail: reserve the largest docs for one trailing direct tile
    tail = []
    for _ in range(TAIL_TILES):
        g = 1
        t = t_even(end - 1)
        while end - g - 1 >= pos and (g + 1) * t <= TILE_COLS:
            g += 1
        end -= g
        tail.insert(0, ("direct", g, t, end, None, g * t))

    v_load = s_load = 0.0
    for _ in range(PRIMER_TILES):
        it = take_direct(pos, end)
        items.append(it)
        v_load += it[5] * R_RED
        pos += it[1]

    while pos < end:
        # superblock
        g = 1
        while pos + g < end and (g + 1) * t_8(pos + g) <= SB_COLS:
            g += 1
        t = t_8(pos + g - 1)
        it = ("sb", g, t, pos, None, g * t)
        items.append(it)
        v_load += g * t * R_TREE
        s_load += g * t * R_COPY
        pos += g
        # plug a direct tile when Vector is clearly behind Scalar
        if pos < end and s_load - v_load > 1000.0:
            it = take_direct(pos, end)
            items.append(it)
            v_load += it[5] * R_RED
            pos += it[1]

    items.extend(tail)

    col = 0
    out = []
    for kind, g, t, doc_off, _, cols in items:
        out.append((kind, g, t, doc_off, col, cols))
        col += cols
    return out, col


def _build_program(items, c_total, qt_cols):
    import concourse.bacc as bacc
    import concourse.mybir as mybir
    import concourse.tile as tile

    f32 = mybir.dt.float32
    bf16 = mybir.dt.bfloat16
    fp16 = mybir.dt.float16
    AL = mybir.AluOpType
    ACTF = mybir.ActivationFunctionType

    nc = bacc.Bacc("TRN2", target_bir_lowering=False)

    dn = nc.dram_tensor("dn", [K, c_total], bf16, kind="ExternalInput")
    qt = nc.dram_tensor("qt", [K, qt_cols], bf16, kind="ExternalInput")
    oh = nc.dram_tensor("ohw", [128, B], fp16, kind="ExternalInput")
    sc = nc.dram_tensor("scores", [D, B], f32, kind="ExternalOutput")

    with ExitStack() as ctx:
        tc = ctx.enter_context(tile.TileContext(nc))
        const = ctx.enter_context(tc.tile_pool(name="const", bufs=1))
        trees = ctx.enter_context(tc.tile_pool(name="trees", bufs=2))
        pssim = ctx.enter_context(tc.tile_pool(name="pssim", bufs=3, space="PSUM"))
        psout = ctx.enter_context(tc.tile_pool(name="psout", bufs=1, space="PSUM"))

        # small constants ride the Scalar-engine HW DGE queue so the Sync
        # queue's first descriptor is the primer doc tile
        qt_sb = const.tile([K, qt_cols], bf16)
        nc.scalar.dma_start(qt_sb, qt[:, :])
        oh_sb = const.tile([128, B], fp16)
        nc.scalar.dma_start(oh_sb, oh[:, :])

        mx = const.tile([128, D], fp16)

        # per-piece SBUF chunk tiles + DMAs so matmuls gate on 0.26 MB, not
        # whole superblocks. A few early descriptors ride the Scalar queue
        # (it is idle until its first PSUM copy) to overlap issue cost.
        chunk = {}
        n_sc = 0
        for i, (kind, g, t, doc_off, col_off, cols) in enumerate(items):
            pcs = []
            for p0 in range(0, cols, TILE_COLS):
                psz = min(TILE_COLS, cols - p0)
                ct = const.tile([K, psz], bf16, name=f"dc{i}_{p0}",
                                tag=f"dc{i}_{p0}")
                if 2 <= i + n_sc <= 5 and n_sc < 4:
                    eng = nc.scalar
                    n_sc += 1
                else:
                    eng = nc.sync
                eng.dma_start(ct, dn[:, col_off + p0:col_off + p0 + psz])
                pcs.append(ct)
            chunk[i] = pcs

        def emit_matmuls(pcs):
            """PE matmuls for one item; returns list of (psum_tile, lo, sz)."""
            pieces = []
            p0 = 0
            for ct in pcs:
                psz = ct.shape[1]
                sim = pssim.tile([128, TILE_COLS], f32, tag="sim")
                for s in range(0, psz, MM_CHUNK):
                    e = min(s + MM_CHUNK, psz)
                    nc.tensor.matmul(
                        sim[:, s:e], qt_sb, ct[:, s:e],
                        start=True, stop=True, skip_group_check=True,
                    )
                pieces.append((sim, p0, psz))
                p0 += psz
            return pieces

        for i, (kind, g, t, doc_off, col_off, cols) in enumerate(items):
            pieces = emit_matmuls(chunk[i])
            if kind == "direct":
                (sim, _, psz) = pieces[0]
                nc.vector.tensor_reduce(
                    mx[:, doc_off:doc_off + g],
                    sim[:, :psz].rearrange("p (d t) -> p d t", d=g),
                    axis=mybir.AxisListType.X, op=AL.max,
                )
            else:
                arena = trees.tile([128, cols], fp16, name=f"arena{i}",
                                   tag="arena")
                for (sim, p0, psz) in pieces:
                    nc.scalar.activation(
                        arena[:, p0:p0 + psz], sim[:, :psz], ACTF.Copy,
                        bias=0.0, scale=1.0,
                    )
                h = cols // 2
                t1 = trees.tile([128, h], fp16, name=f"t1_{i}", tag="t1")
                nc.vector.tensor_max(t1, arena[:, :h], arena[:, h:])
                t2 = trees.tile([128, h // 2], fp16, name=f"t2_{i}", tag="t2")
                nc.vector.tensor_max(t2, t1[:, :h // 2], t1[:, h // 2:])
                t3 = trees.tile([128, h // 4], fp16, name=f"t3_{i}", tag="t3")
                nc.vector.tensor_max(t3, t2[:, :h // 4], t2[:, h // 4:])
                nc.vector.tensor_reduce(
                    mx[:, doc_off:doc_off + g],
                    t3.rearrange("p (d t) -> p d t", d=g),
                    axis=mybir.AxisListType.X, op=AL.max,
                )

        scp = psout.tile([D, B], f32, tag="scp")
        nc.tensor.matmul(scp, mx, oh_sb, start=True, stop=True,
                         skip_group_check=True)
        scsb = const.tile([D, B], f32)
        nc.vector.tensor_copy(scsb, scp)
        nc.scalar.dma_start(sc[:, :], scsb)

    nc.finalize()
    return nc


def _get_program(items, c_total, qt_cols):
    key = (tuple(items), c_total, qt_cols)
    if key not in _CACHE:
        _CACHE[key] = _build_program(items, c_total, qt_cols)
    return _CACHE[key]


def kernel(q_hidden, q_mask, d_hidden, d_mask):
    global LAST_EXEC_NS
    import ml_dtypes
    from concourse.bass_utils import run_bass_kernel_spmd

    bf16 = ml_dtypes.bfloat16
    q_hidden = np.asarray(q_hidden, dtype=np.float32)
    q_mask = np.asarray(q_mask)
    d_hidden = np.asarray(d_hidden, dtype=np.float32)
    d_mask = np.asarray(d_mask)

    # ---- doc prep ----
    dnorm = np.sqrt(np.sum(d_hidden * d_hidden, axis=-1, keepdims=True))
    dhat = d_hidden / np.maximum(dnorm, EPS)               # [N, Ld, K]
    act_d = d_mask > 0
    n_act = act_d.sum(axis=1).astype(np.int64)             # [N]

    orders = []
    sorted_nact = np.empty((NCORES, D), np.int64)
    for c in range(NCORES):
        na = n_act[c * D:(c + 1) * D]
        o = np.argsort(na, kind="stable")
        orders.append(o)
        sorted_nact[c] = na[o]

    tpos = np.maximum(sorted_nact.max(axis=0), 2)
    items, c_total = _plan(tpos)

    # token->column maps per core
    tok_idx = np.zeros((NCORES, c_total), np.int64)
    doc_idx = np.zeros((NCORES, c_total), np.int64)
    for c in range(NCORES):
        for (kind, g, t, doc_off, col_off, cols) in items:
            for j in range(g):
                doc = c * D + int(orders[c][doc_off + j])
                ti = np.nonzero(act_d[doc])[0]
                reps = np.zeros(t, np.int64)
                if len(ti):
                    k = min(len(ti), t)
                    reps[:k] = ti[:k]
                    reps[k:] = ti[0]
                if kind == "direct":
                    doc_idx[c, col_off + j * t:col_off + (j + 1) * t] = doc
                    tok_idx[c, col_off + j * t:col_off + (j + 1) * t] = reps
                else:
                    # bit-block: col(b, d, jj) = b*(g*t/8) + d*(t/8) + jj,
                    # token index = jj*8 + b
                    t8 = t // 8
                    for b in range(8):
                        s = col_off + b * g * t8 + j * t8
                        tok_idx[c, s:s + t8] = reps[b::8][:t8]
                        doc_idx[c, s:s + t8] = doc
    dn_all = dhat[doc_idx, tok_idx]                        # [NCORES, C, K]
    dn_all = np.ascontiguousarray(dn_all.transpose(0, 2, 1)).astype(bf16)

    # ---- query prep ----
    qf = q_hidden.reshape(B * Lq, K)
    act = np.nonzero(q_mask.reshape(-1) > 0)[0]
    nq = len(act)
    assert nq <= 128, f"active q tokens {nq} > 128 unsupported"
    QT = 128
    qpack = np.ones((QT, K), np.float32)
    qpack[:nq] = qf[act]
    qn = np.sqrt(np.sum(qpack * qpack, axis=-1))
    qinv = 1.0 / np.maximum(qn, EPS)
    ohw = np.zeros((QT, B), np.float32)
    ohw[np.arange(nq), act // Lq] = qinv[:nq]
    ohw = ohw.astype(np.float16)
    qt_p = np.ascontiguousarray(qpack.T).astype(bf16)      # [K, QT]

    in_maps = [
        {"dn": dn_all[c], "qt": qt_p, "ohw": ohw} for c in range(NCORES)
    ]

    nc = _get_program(items, c_total, QT)
    kw = {}
    if os.environ.get("KRN_TMPDIR"):
        kw["tmpdir"] = os.environ["KRN_TMPDIR"]
    br = run_bass_kernel_spmd(nc, in_maps, core_ids=list(range(NCORES)), **kw)
    if br.exec_time_ns is not None:
        LAST_EXEC_NS = br.exec_time_ns

    scores = np.empty((B, N), np.float32)
    for c in range(NCORES):
        out_c = br.results[c]["scores"]                    # [D, B] sorted order
        scores[:, c * D + orders[c]] = out_c.T
    dead = np.nonzero(n_act == 0)[0]
    if len(dead):
        nactq = (q_mask > 0).sum(axis=1).astype(np.float32)
        scores[:, dead] = (NEG * nactq)[:, None]
    return scores


if __name__ == "__main__":
    tpos = np.full(D, 129)
    items, c_total = _plan(tpos)
    for it in items:
        print(it)
    print("c_total:", c_total)
    nc = _get_program(tuple(items), c_total, 128)
    print("program built OK; instructions:",
          sum(len(bb.instructions) for bb in nc.main_func.blocks))


# revision 18
# speedup vs baseline: 1.0871x; 1.0238x over previous
"""ColBERT MaxSim retrieval kernel for Trainium2 (8 NeuronCores).

scores[b, n] = sum_{q active} max_{t active} cos(q_hidden[b,q], d_hidden[n,t])

Host: d tokens are masked, L2-normalized, compacted (only active tokens
kept), bf16-cast, and laid out per a schedule that the device program is
specialized to. Query active tokens are packed raw into 128 slots; query
inverse norms fold into the final scores matmul weights.

Device (per core, 128 docs): stream dn through PE (stationary qT, bf16,
512-col matmuls into PSUM). The token-max runs on two lanes:
  - direct:  Vector tensor_reduce [128, G, T] straight from PSUM
  - offload: Scalar copies PSUM->SBUF fp16, then Vector runs a contiguous
    TT-max tree (2 elems/cycle in fp16 2x mode). Host lays the offloaded
    superblocks out in 8-way bit-blocks (col(b,d,j) = b*G*T/8 + d*T/8 + j,
    token = j*8+b) so all three halving levels are single-segment
    contiguous TTs; the tail reduce sees a doc-major [128, G, T/8] view.
Final: PE matmul mx.T @ (onehot*qinv) -> [doc, batch] scores, DMA out.

Stream order = [primer direct tiles (smallest docs), superblocks,
direct tail tiles (largest docs)] so Vector starts early and the last
DMA arrivals take the shortest path. DMA: first descriptor is the small
primer tile; qt/ohw/output ride the Scalar-engine HW queue.
"""

import os
import sys
from contextlib import ExitStack

import numpy as np

sys.path.insert(0, "/opt/trn_rl_repo")

# ---- problem constants (hardcoded per contest contract) ----
B, Lq, N, Ld, K = 8, 32, 1024, 256, 128
NCORES = 8
D = N // NCORES          # 128 docs per core
EPS = 1e-8
NEG = -100.0

TILE_COLS = 1024         # PSUM tile: 2 banks, triple-buffered
MM_CHUNK = 512
PRIMER_TILES = 2         # leading direct tiles (pipeline fill)
TAIL_TILES = 1           # trailing direct tiles (largest-T docs)
SB_COLS = 5120           # superblock target size (5 PSUM tiles)
SB_LAST_COLS = 2048      # the last superblock is kept small (short end chain)
FP8 = True               # dn stream in fp8e4m3 (qt stays bf16), scale folded
FP8_SCALE = 16.0

# measured per-column engine rates (ns) for load balancing
R_TREE = 0.72            # Vector TT tree + tail, per sb col
R_RED = 1.19             # Vector direct reduce, per col (incl per-op init)
R_COPY = 1.08            # Scalar PSUM->SBUF copy, per col (1024-col pieces)

_CACHE = {}
LAST_EXEC_NS = None


def _plan(tpos):
    """tpos: [D] per-sorted-position token requirement (cross-core max).
    Returns (items, c_total). Items in stream order:
      ('direct', G, T, doc_off, col_off, cols)
      ('sb',     G, T, doc_off, col_off, cols)   # cols = G*T, bit-block laid
    """
    items = []
    pos = 0
    end = D

    def t_even(p):
        return int((tpos[p] + 1) // 2 * 2)

    def t_8(p):
        return int((tpos[p] + 7) // 8 * 8)

    def take_direct(pos, end_limit):
        g = 1
        while pos + g < end_limit and (g + 1) * t_even(pos + g) <= TILE_COLS:
            g += 1
        t = t_even(pos + g - 1)
        return ("direct", g, t, pos, None, g * t)

    # tail: reserve the largest docs for one trailing direct tile
    tail = []
    for _ in range(TAIL_TILES):
        g = 1
        t = t_even(end - 1)
        while end - g - 1 >= pos and (g + 1) * t <= TILE_COLS:
            g += 1
        end -= g
        tail.insert(0, ("direct", g, t, end, None, g * t))

    # reserve a small final superblock (short end-of-stream chain) from the
    # largest remaining docs
    g_last = 1
    while end - g_last - 1 >= pos and (g_last + 1) * t_8(end - 1) <= SB_LAST_COLS:
        g_last += 1
    end -= g_last
    sb_last = ("sb", g_last, t_8(end + g_last - 1), end, None,
               g_last * t_8(end + g_last - 1))

    v_load = s_load = 0.0
    for _ in range(PRIMER_TILES):
        it = take_direct(pos, end)
        items.append(it)
        v_load += it[5] * R_RED
        pos += it[1]

    while pos < end:
        # superblock
        g = 1
        while pos + g < end and (g + 1) * t_8(pos + g) <= SB_COLS:
            g += 1
        t = t_8(pos + g - 1)
        it = ("sb", g, t, pos, None, g * t)
        items.append(it)
        v_load += g * t * R_TREE
        s_load += g * t * R_COPY
        pos += g
        # plug a direct tile when Vector is clearly behind Scalar
        if pos < end and s_load - v_load > 1000.0:
            it = take_direct(pos, end)
            items.append(it)
            v_load += it[5] * R_RED
            pos += it[1]

    items.append(sb_last)
    items.extend(tail)

    col = 0
    out = []
    for kind, g, t, doc_off, _, cols in items:
        out.append((kind, g, t, doc_off, col, cols))
        col += cols
    return out, col


def _build_program(items, c_total, qt_cols):
    import concourse.bacc as bacc
    import concourse.mybir as mybir
    import concourse.tile as tile

    f32 = mybir.dt.float32
    bf16 = mybir.dt.bfloat16
    fp16 = mybir.dt.float16
    AL = mybir.AluOpType
    ACTF = mybir.ActivationFunctionType

    nc = bacc.Bacc("TRN2", target_bir_lowering=False)

    ddt = mybir.dt.float8e4 if FP8 else bf16
    dn = nc.dram_tensor("dn", [K, c_total], ddt, kind="ExternalInput")
    qt = nc.dram_tensor("qt", [K, qt_cols], bf16, kind="ExternalInput")
    oh = nc.dram_tensor("ohw", [128, B], fp16, kind="ExternalInput")
    sc = nc.dram_tensor("scores", [D, B], f32, kind="ExternalOutput")

    with ExitStack() as ctx:
        tc = ctx.enter_context(tile.TileContext(nc))
        const = ctx.enter_context(tc.tile_pool(name="const", bufs=1))
        trees = ctx.enter_context(tc.tile_pool(name="trees", bufs=2))
        pssim = ctx.enter_context(tc.tile_pool(name="pssim", bufs=3, space="PSUM"))
        psout = ctx.enter_context(tc.tile_pool(name="psout", bufs=1, space="PSUM"))

        # small constants ride the Scalar-engine HW DGE queue so the Sync
        # queue's first descriptor is the primer doc tile
        qt_sb = const.tile([K, qt_cols], bf16)
        nc.scalar.dma_start(qt_sb, qt[:, :])
        oh_sb = const.tile([128, B], fp16)
        nc.scalar.dma_start(oh_sb, oh[:, :])

        mx = const.tile([128, D], fp16)

        # per-piece SBUF chunk tiles + DMAs so matmuls gate on one PSUM
        # tile's worth of data. Early descriptors alternate between the
        # Sync and Scalar HW queues (Scalar's queue is idle until its
        # first PSUM copy) so the 4-deep descriptor rings don't stall.
        chunk = {}
        k_pc = 0
        for i, (kind, g, t, doc_off, col_off, cols) in enumerate(items):
            pcs = []
            for p0 in range(0, cols, TILE_COLS):
                psz = min(TILE_COLS, cols - p0)
                ct = const.tile([K, psz], ddt, name=f"dc{i}_{p0}",
                                tag=f"dc{i}_{p0}")
                eng = nc.scalar if (k_pc % 2 == 1 and k_pc < 8) else nc.sync
                k_pc += 1
                eng.dma_start(ct, dn[:, col_off + p0:col_off + p0 + psz])
                pcs.append(ct)
            chunk[i] = pcs

        def emit_matmuls(pcs):
            """PE matmuls for one item; returns list of (psum_tile, lo, sz)."""
            pieces = []
            p0 = 0
            for ct in pcs:
                psz = ct.shape[1]
                sim = pssim.tile([128, TILE_COLS], f32, tag="sim")
                for s in range(0, psz, MM_CHUNK):
                    e = min(s + MM_CHUNK, psz)
                    nc.tensor.matmul(
                        sim[:, s:e], qt_sb, ct[:, s:e],
                        start=True, stop=True, skip_group_check=True,
                    )
                pieces.append((sim, p0, psz))
                p0 += psz
            return pieces

        def emit_tree(i, g, doc_off, cols, arena):
            h = cols // 2
            t1 = trees.tile([128, h], fp16, name=f"t1_{i}", tag="t1")
            nc.vector.tensor_max(t1, arena[:, :h], arena[:, h:])
            t2 = trees.tile([128, h // 2], fp16, name=f"t2_{i}", tag="t2")
            nc.vector.tensor_max(t2, t1[:, :h // 2], t1[:, h // 2:])
            t3 = trees.tile([128, h // 4], fp16, name=f"t3_{i}", tag="t3")
            nc.vector.tensor_max(t3, t2[:, :h // 4], t2[:, h // 4:])
            nc.vector.tensor_reduce(
                mx[:, doc_off:doc_off + g],
                t3.rearrange("p (d t) -> p d t", d=g),
                axis=mybir.AxisListType.X, op=AL.max,
            )

        last_sb = max(i for i, it in enumerate(items) if it[0] == "sb")
        deferred = None
        for i, (kind, g, t, doc_off, col_off, cols) in enumerate(items):
            pieces = emit_matmuls(chunk[i])
            if kind == "direct":
                (sim, _, psz) = pieces[0]
                nc.vector.tensor_reduce(
                    mx[:, doc_off:doc_off + g],
                    sim[:, :psz].rearrange("p (d t) -> p d t", d=g),
                    axis=mybir.AxisListType.X, op=AL.max,
                )
            else:
                arena = trees.tile([128, cols], fp16, name=f"arena{i}",
                                   tag="arena")
                for (sim, p0, psz) in pieces:
                    nc.scalar.activation(
                        arena[:, p0:p0 + psz], sim[:, :psz], ACTF.Copy,
                        bias=0.0, scale=1.0,
                    )
                if i == last_sb:
                    # the final sb's tree runs after the trailing direct
                    # reduces so Vector's in-order queue never stalls on it
                    deferred = (i, g, doc_off, cols, arena)
                else:
                    emit_tree(i, g, doc_off, cols, arena)
        if deferred is not None:
            emit_tree(*deferred)

        scp = psout.tile([D, B], f32, tag="scp")
        nc.tensor.matmul(scp, mx, oh_sb, start=True, stop=True,
                         skip_group_check=True)
        scsb = const.tile([D, B], f32)
        nc.scalar.activation(scsb, scp, ACTF.Copy, bias=0.0, scale=1.0)
        nc.scalar.dma_start(sc[:, :], scsb)

    nc.finalize()
    return nc


def _get_program(items, c_total, qt_cols):
    key = (tuple(items), c_total, qt_cols)
    if key not in _CACHE:
        _CACHE[key] = _build_program(items, c_total, qt_cols)
    return _CACHE[key]


def kernel(q_hidden, q_mask, d_hidden, d_mask):
    global LAST_EXEC_NS
    import ml_dtypes
    from concourse.bass_utils import run_bass_kernel_spmd

    bf16 = ml_dtypes.bfloat16
    q_hidden = np.asarray(q_hidden, dtype=np.float32)
    q_mask = np.asarray(q_mask)
    d_hidden = np.asarray(d_hidden, dtype=np.float32)
    d_mask = np.asarray(d_mask)

    # ---- doc prep ----
    dnorm = np.sqrt(np.sum(d_hidden * d_hidden, axis=-1, keepdims=True))
    dhat = d_hidden / np.maximum(dnorm, EPS)               # [N, Ld, K]
    act_d = d_mask > 0
    n_act = act_d.sum(axis=1).astype(np.int64)             # [N]

    orders = []
    sorted_nact = np.empty((NCORES, D), np.int64)
    for c in range(NCORES):
        na = n_act[c * D:(c + 1) * D]
        o = np.argsort(na, kind="stable")
        orders.append(o)
        sorted_nact[c] = na[o]

    tpos = np.maximum(sorted_nact.max(axis=0), 2)
    items, c_total = _plan(tpos)

    # token->column maps per core
    tok_idx = np.zeros((NCORES, c_total), np.int64)
    doc_idx = np.zeros((NCORES, c_total), np.int64)
    for c in range(NCORES):
        for (kind, g, t, doc_off, col_off, cols) in items:
            for j in range(g):
                doc = c * D + int(orders[c][doc_off + j])
                ti = np.nonzero(act_d[doc])[0]
                reps = np.zeros(t, np.int64)
                if len(ti):
                    k = min(len(ti), t)
                    reps[:k] = ti[:k]
                    reps[k:] = ti[0]
                if kind == "direct":
                    doc_idx[c, col_off + j * t:col_off + (j + 1) * t] = doc
                    tok_idx[c, col_off + j * t:col_off + (j + 1) * t] = reps
                else:
                    # bit-block: col(b, d, jj) = b*(g*t/8) + d*(t/8) + jj,
                    # token index = jj*8 + b
                    t8 = t // 8
                    for b in range(8):
                        s = col_off + b * g * t8 + j * t8
                        tok_idx[c, s:s + t8] = reps[b::8][:t8]
                        doc_idx[c, s:s + t8] = doc
    dn_all = dhat[doc_idx, tok_idx]                        # [NCORES, C, K]
    dn_all = np.ascontiguousarray(dn_all.transpose(0, 2, 1))
    if FP8:
        dn_all = (dn_all * FP8_SCALE).astype(ml_dtypes.float8_e4m3)
    else:
        dn_all = dn_all.astype(bf16)

    # ---- query prep ----
    qf = q_hidden.reshape(B * Lq, K)
    act = np.nonzero(q_mask.reshape(-1) > 0)[0]
    nq = len(act)
    assert nq <= 128, f"active q tokens {nq} > 128 unsupported"
    QT = 128
    qpack = np.ones((QT, K), np.float32)
    qpack[:nq] = qf[act]
    qn = np.sqrt(np.sum(qpack * qpack, axis=-1))
    qinv = 1.0 / np.maximum(qn, EPS)
    ohw = np.zeros((QT, B), np.float32)
    ohw[np.arange(nq), act // Lq] = qinv[:nq]
    if FP8:
        ohw /= FP8_SCALE
    ohw = ohw.astype(np.float16)
    qt_p = np.ascontiguousarray(qpack.T).astype(bf16)      # [K, QT]

    in_maps = [
        {"dn": dn_all[c], "qt": qt_p, "ohw": ohw} for c in range(NCORES)
    ]

    nc = _get_program(items, c_total, QT)
    kw = {}
    if os.environ.get("KRN_TMPDIR"):
        kw["tmpdir"] = os.environ["KRN_TMPDIR"]
    br = run_bass_kernel_spmd(nc, in_maps, core_ids=list(range(NCORES)), **kw)
    if br.exec_time_ns is not None:
        LAST_EXEC_NS = br.exec_time_ns

    scores = np.empty((B, N), np.float32)
    for c in range(NCORES):
        out_c = br.results[c]["scores"]                    # [D, B] sorted order
        scores[:, c * D + orders[c]] = out_c.T
    dead = np.nonzero(n_act == 0)[0]
    if len(dead):
        nactq = (q_mask > 0).sum(axis=1).astype(np.float32)
        scores[:, dead] = (NEG * nactq)[:, None]
    return scores


if __name__ == "__main__":
    tpos = np.full(D, 129)
    items, c_total = _plan(tpos)
    for it in items:
        print(it)
    print("c_total:", c_total)
    nc = _get_program(tuple(items), c_total, 128)
    print("program built OK; instructions:",
          sum(len(bb.instructions) for bb in nc.main_func.blocks))
